# revision 9
# baseline (speedup 1.0000x reference)
"""BOCPD-GPTS nlml kernel for 8 Trainium2 NeuronCores.

Math: in the reference, the run-length posterior r is renormalized every
step, so sum(r)==1 entering each step and ssum == upm_t exactly.  Hence
nlml = sum_t upm_t ... UNLESS some upm_t underflows to 0 in f32 (it does,
at t=113 for the reference inputs): then new/ssum = 0/0 = NaN poisons r
and every later ssum, so nlml == NaN whenever any upm_t == 0 for t < T.

So the kernel computes the 2048 independent GP predictive densities
(127x127 masked-RBF Cholesky + 2 triangular solves each) data-parallel:
8 cores x 2 lane-batches x 128 lanes, one window system per SBUF lane.
Host glue only builds windows/masks (index bookkeeping), applies the
sum / NaN rule, and reshapes.
"""

import numpy as np

try:
    from concourse import bass, mybir
    from concourse.tile import TileContext
    from concourse.bass_utils import run_bass_kernel_spmd
except ImportError:  # pragma: no cover
    import sys

    sys.path.insert(0, "/opt/trn_rl_repo")
    from concourse import bass, mybir
    from concourse.tile import TileContext
    from concourse.bass_utils import run_bass_kernel_spmd

T = 2048
WIN = 128
W = 127  # training-window length (window_size - 1)
P = 128  # SBUF partitions = lane-batch size
B2 = 2  # lane-batches per core (256 t-values / core)
NCORES = 8

F32 = mybir.dt.float32
AF = mybir.ActivationFunctionType
OP = mybir.AluOpType


def _build_nc(ell2: float, sig2: float):
    """Raw Block-style Bass kernel (Tile's redundant same-engine waits trip a
    one-sync-wait-per-instruction limit in this walrus build, so semaphores
    are managed manually; every wait is a standalone wait_ge instruction)."""
    nc = bass.Bass("TRN2", target_bir_lowering=False, debug=False)

    # One packed input: rows 0=xw 1=ywm 2=ms 3=m2 4=dg 5=[xs,ys,0...]
    pk_d = nc.dram_tensor("pk_in", [P, B2, 6, W], F32, kind="ExternalInput").ap()
    out_d = nc.dram_tensor("upm_out", [P, B2], F32, kind="ExternalOutput").ap()

    neg_half_inv_ell2 = float(-0.5 / ell2)
    inv_2pi = float(1.0 / (2.0 * np.pi))

    def sb(name, shape):
        return nc.alloc_sbuf_tensor(name, list(shape), F32).ap()

    raw = sb("raw", [P, B2, 6, W])
    K = sb("K", [P, B2, W, W])
    tmp = sb("tmp", [P, B2, 4064])
    sred = sb("sred", [P, B2, W])
    invd = sb("invd", [P, B2, W])
    xw = sb("xw", [P, B2, W])
    ms = sb("ms", [P, B2, W])
    m2 = sb("m2", [P, B2, W])
    dg = sb("dg", [P, B2, W])
    b = sb("b", [P, B2, 2, W])
    xs = sb("xs", [P, B2])
    ys = sb("ys", [P, B2])
    recd = sb("recd", [P, B2])
    mu = sb("mu", [P, B2])
    q = sb("q", [P, B2])
    vinv = sb("vinv", [P, B2])
    rs = sb("rs", [P, B2])
    dv = sb("dv", [P, B2])
    upm = sb("upm", [P, B2])

    with (
        nc.Block() as block,
        nc.semaphore("dsem") as dsem,
        nc.semaphore("vsem") as vsem,
        nc.semaphore("asem") as asem,
    ):
        # DVE writes are posted: every RAW/WAR hazard (same-engine included)
        # needs a semaphore wait, and this walrus build fits only ONE sync
        # wait per instruction — so every wait is a standalone wait_ge and
        # every compute op incs its engine's sem. Ticks are tracked in
        # python; ACT ticks are the analytic table SqK=1 ExpK=2 Sqkx=3
        # Expkx=4 sqrt_j=5+j SqRs=132 ExpDv=133.
        vt = {"n": 0}
        vticks = {}

        def A_SQRT(j):
            return 5 + j

        A_EXPK, A_EXPKX, A_SQRS, A_EXPDV = 2, 4, 132, 133

        @block.vector
        def _(v):
            seen = {"v": 0, "a": 0}

            def wv(t):
                if t > seen["v"]:
                    v.wait_ge(vsem, t)
                    seen["v"] = t

            def wa(t):
                if t > seen["a"]:
                    v.wait_ge(asem, t)
                    seen["a"] = t

            def op(inst, key=None):
                inst.then_inc(vsem)
                vt["n"] += 1
                if key is not None:
                    vticks[key] = vt["n"]
                return vt["n"]

            v.wait_ge(dsem, 16)
            op(v.tensor_scalar_add(ms[:], raw[:, :, 2, :], 0.0))
            op(v.tensor_scalar_add(m2[:], raw[:, :, 3, :], 0.0))
            t_dg = op(v.tensor_scalar_add(dg[:], raw[:, :, 4, :], 0.0))
            op(v.tensor_scalar_add(ys[:], raw[:, :, 5, 1], 0.0))
            t_b0 = op(v.tensor_scalar_add(b[:, :, 0, :], raw[:, :, 1, :], 0.0))
            t_xw = op(v.tensor_scalar_add(xw[:], raw[:, :, 0, :], 0.0))
            t_xs = op(v.tensor_scalar_add(xs[:], raw[:, :, 5, 0], 0.0))

            xi = xw.unsqueeze(3).broadcast_to([P, B2, W, W])
            xj = xw.unsqueeze(2).broadcast_to([P, B2, W, W])
            wv(t_xw)
            op(v.tensor_tensor(out=K[:], in0=xi, in1=xj, op=OP.subtract), "dsub")
            kx = b[:, :, 1, :]
            xsb = xs.unsqueeze(2).broadcast_to([P, B2, W])
            wv(t_xs)
            op(v.tensor_tensor(out=kx, in0=xw[:], in1=xsb, op=OP.subtract), "kxsub")
            mi = ms.unsqueeze(3).broadcast_to([P, B2, W, W])
            mj = ms.unsqueeze(2).broadcast_to([P, B2, W, W])
            wa(A_EXPK)
            tK = op(v.tensor_mul(out=K[:], in0=K[:], in1=mi))
            wv(tK)
            tK = op(v.tensor_mul(out=K[:], in0=K[:], in1=mj))
            kdiag = K[:].rearrange("p b i j -> p b (i j)")[:, :, 0 : W * W : W + 1]
            wv(tK)
            tK = op(v.tensor_tensor(out=kdiag, in0=kdiag, in1=dg[:], op=OP.add))
            wa(A_EXPKX)
            t_kx = op(v.tensor_mul(out=kx, in0=kx, in1=m2[:]))

            # ---- left-looking Cholesky ----
            for j in range(W):
                nj = W - j
                kcol = K[:, :, j:W, j]
                if j > 0:
                    tv = tmp[:, :, 0 : nj * j].rearrange(
                        "p b (r k) -> p b r k", r=nj, k=j
                    )
                    lrows = K[:, :, j:W, 0:j]
                    ljrow = K[:, :, j, 0:j].unsqueeze(2).broadcast_to([P, B2, nj, j])
                    wv(tK)
                    tM = op(v.tensor_mul(out=tv, in0=lrows, in1=ljrow))
                    wv(tM)
                    tR = op(v.tensor_reduce(
                        out=sred[:, :, 0:nj], in_=tv,
                        axis=mybir.AxisListType.X, op=OP.add,
                    ))
                    wv(tR)
                    tS = op(v.tensor_tensor(
                        out=kcol, in0=kcol, in1=sred[:, :, 0:nj], op=OP.subtract
                    ))
                else:
                    tS = tK
                wv(tS)
                if j > 0:
                    wa(A_SQRT(j - 1))  # WAR: ACT sqrt_{j-1} read recd
                op(v.reciprocal(out=recd[:], in_=K[:, :, j, j]), ("recip", j))
                if j < W - 1:
                    kcol2 = K[:, :, j + 1 : W, j]
                    ib = invd[:, :, j].unsqueeze(2).broadcast_to([P, B2, nj - 1])
                    wa(A_SQRT(j))
                    tK = op(v.tensor_mul(out=kcol2, in0=kcol2, in1=ib))

            # ---- joint forward solve ----
            wa(A_SQRT(W - 1))
            tB = max(t_b0, t_kx)
            for j in range(W):
                bj = b[:, :, :, j]
                ib = invd[:, :, j].unsqueeze(2).broadcast_to([P, B2, 2])
                wv(tB)
                tV = op(v.tensor_mul(out=bj, in0=bj, in1=ib))
                if j < W - 1:
                    njj = W - 1 - j
                    tv = tmp[:, :, 0 : 2 * njj].rearrange(
                        "p b (r i) -> p b r i", r=2, i=njj
                    )
                    lcol = (
                        K[:, :, j + 1 : W, j]
                        .unsqueeze(2).broadcast_to([P, B2, 2, njj])
                    )
                    vb = bj.unsqueeze(3).broadcast_to([P, B2, 2, njj])
                    wv(tV)
                    tM = op(v.tensor_mul(out=tv, in0=lcol, in1=vb))
                    wv(tM)
                    tB = op(v.tensor_tensor(
                        out=b[:, :, :, j + 1 : W], in0=b[:, :, :, j + 1 : W],
                        in1=tv, op=OP.subtract,
                    ))
                else:
                    tB = tV

            # ---- mu, var, upm ----
            A_ = b[:, :, 1, :]
            V_ = b[:, :, 0, :]
            pv = tmp[:, :, 0:W]
            wv(tB)
            t1 = op(v.tensor_mul(out=pv, in0=A_, in1=V_))
            wv(t1)
            t1 = op(v.tensor_reduce(out=mu[:], in_=pv, axis=mybir.AxisListType.X, op=OP.add))
            wv(t1)
            t1 = op(v.tensor_mul(out=pv, in0=A_, in1=A_))
            wv(t1)
            t1 = op(v.tensor_reduce(out=q[:], in_=pv, axis=mybir.AxisListType.X, op=OP.add))
            wv(t1)
            t1 = op(v.tensor_scalar(
                out=q[:], in0=q[:], scalar1=-1.0, scalar2=float(sig2),
                op0=OP.mult, op1=OP.add,
            ))
            wv(t1)
            t1 = op(v.reciprocal(out=vinv[:], in_=q[:]))
            wv(t1)
            t_rs = op(v.tensor_scalar(
                out=rs[:], in0=vinv[:], scalar1=inv_2pi, scalar2=None, op0=OP.mult
            ))
            t1 = op(v.tensor_tensor(out=dv[:], in0=ys[:], in1=mu[:], op=OP.subtract))
            wv(t1)
            t1 = op(v.tensor_mul(out=dv[:], in0=dv[:], in1=dv[:]))
            wv(t1)
            t_dv = op(v.tensor_mul(out=dv[:], in0=dv[:], in1=vinv[:]), "dvfin")
            wa(A_EXPDV)
            op(v.tensor_mul(out=upm[:], in0=dv[:], in1=rs[:]), "upm")

        @block.scalar
        def _(s):
            aseen = {"v": 0}

            def wv(t):
                if t > aseen["v"]:
                    s.wait_ge(vsem, t)
                    aseen["v"] = t

            wv(vticks["dsub"])
            s.activation(out=K[:], in_=K[:], func=AF.Square).then_inc(asem)
            s.wait_ge(asem, 1)
            s.activation(
                out=K[:], in_=K[:], func=AF.Exp, scale=neg_half_inv_ell2
            ).then_inc(asem)
            kx = b[:, :, 1, :]
            wv(vticks["kxsub"])
            s.activation(out=kx, in_=kx, func=AF.Square).then_inc(asem)
            s.wait_ge(asem, 3)
            s.activation(
                out=kx, in_=kx, func=AF.Exp, scale=neg_half_inv_ell2
            ).then_inc(asem)
            for j in range(W):
                wv(vticks[("recip", j)])
                s.activation(out=invd[:, :, j], in_=recd[:], func=AF.Sqrt).then_inc(
                    asem
                )
            wv(vticks["dvfin"])
            s.activation(out=rs[:], in_=rs[:], func=AF.Sqrt).then_inc(asem)
            s.wait_ge(asem, 132)
            s.activation(out=dv[:], in_=dv[:], func=AF.Exp, scale=-0.5).then_inc(asem)

        @block.sync
        def _(sync):
            sync.dma_start(out=raw[:], in_=pk_d).then_inc(dsem, 16)
            sync.wait_ge(vsem, vticks["upm"])
            sync.dma_start(out=out_d, in_=upm[:]).then_inc(dsem, 16)
            sync.wait_ge(dsem, 32)

    return nc


def _host_inputs(x, y, ell2, sig2, noise):
    """Per-core input dicts. t = 1 + 256*core + 128*b2 + lane."""
    tv = np.arange(1, T + 1)
    sv = np.maximum(tv - WIN, 0)
    leff = np.clip(tv - 1, 1, W)
    idx = sv[:, None] + np.arange(W)[None, :]
    xw = x[idx].astype(np.float32)
    yw = y[idx].astype(np.float32)
    mask = (np.arange(W)[None, :] < leff[:, None]).astype(np.float32)
    ywm = yw * mask
    ms = mask * np.float32(np.sqrt(sig2))
    m2 = mask * np.float32(sig2)
    dg = (np.float32(noise) * mask + (np.float32(1.0) - mask)).astype(np.float32)
    xs = x[tv - 1].astype(np.float32)
    ys = y[tv - 1].astype(np.float32)

    def per_core(arr):
        # [T, ...] -> core-major [NCORES][P, B2, ...]
        a = arr.reshape(NCORES, B2, P, *arr.shape[1:])
        return [np.ascontiguousarray(np.swapaxes(a[c], 0, 1)) for c in range(NCORES)]

    sy = np.zeros((T, W), np.float32)
    sy[:, 0] = xs
    sy[:, 1] = ys
    pk = np.stack([xw, ywm, ms, m2, dg, sy], axis=1)  # [T, 6, W]
    PK = per_core(pk)
    return [{"pk_in": PK[c]} for c in range(NCORES)]


def _finalize(upm_all):
    """Reference semantics: ssum_t == upm_t; a zero upm at t<T poisons r
    with 0/0 -> NaN and every later ssum is NaN."""
    z = np.where(upm_all == 0.0)[0]
    if len(z) and z[0] < T - 1:
        return np.array([np.nan], np.float32)
    if not np.all(np.isfinite(upm_all)):
        return np.array([np.nan], np.float32)
    return np.array([np.sum(upm_all, dtype=np.float32)], np.float32)


def kernel(X, Y, log_lengthscale, log_outputscale, log_noise, hazard, window_size,
           _cache={}):
    x = np.asarray(X, np.float32).reshape(-1)
    y = np.asarray(Y, np.float32).reshape(-1)
    ell2 = float(np.exp(np.float32(2.0) * np.float32(log_lengthscale[0])))
    sig2 = float(np.exp(np.float32(log_outputscale[0])))
    noise = float(np.exp(np.float32(log_noise[0])))

    key = (ell2, sig2)
    if key not in _cache:
        _cache[key] = _build_nc(ell2, sig2)
    nc = _cache[key]

    in_maps = _host_inputs(x, y, ell2, sig2, noise)
    res = run_bass_kernel_spmd(nc, in_maps, list(range(NCORES)))

    upm_all = np.empty(T, np.float32)
    for c in range(NCORES):
        out = np.asarray(res.results[c]["upm_out"])  # [P, B2]
        upm_all[c * P * B2 : (c + 1) * P * B2] = np.swapaxes(out, 0, 1).reshape(-1)
    return _finalize(upm_all)
